# revision 1
# baseline (speedup 1.0000x reference)
"""Trainium2 Bass kernel for nn_DistanceNorm.

Computation (B=64, L=2048, M=256), per batch b:
    px    = x[b].sum(axis=0); px /= px.sum()          (density over M bins)
    mean  = sum(px * rng);  std = sqrt(sum(px*(rng-mean)^2))   rng = arange(M)-127
    u[m]  = clip(rng[m]*std/25.6 + mean + 127, -1, 256)
    out[b,l,m] = lerp of x[b,l,:] at position u[m] (zero outside [0,255])

Key identity: the gather+lerp along M is a matmul with the triangle-kernel
matrix  G[r,m] = relu(1 - |r - u[m]|):   out[b] = x[b] @ G[b].

Per-core program (8 batches per core, batch dim sharded over 8 cores),
software-pipelined 3 deep so the PE never waits on the per-batch stats
scalar chain and the DMA queues never back up behind compute:

  segment i:  [stats matmuls + scalar chain for batch i-1]   (PE + DVE)
              [load batch i+1]                               (SP queue)
              [transpose batch i -> xT, px partials]         (PE + DVE/ACT)
              [u broadcast + build G for batch i-1]          (PE + ACT/DVE)
              [apply G for batch i-2, store batch i-2]       (PE, ACT queue)

  - input/output DMAs use 16KB-contiguous partition lines
    (l = p*16 + lc layout), loads issued on the SP HWDGE queue,
    stores on the Activation HWDGE queue so they never serialize.
  - main gather matmuls run in float32r: 1 cycle/row at 256-row moving
    dim vs 4 for float32 (the 2-term triangle-kernel sum keeps the
    precision loss ~1e-5).
"""

from contextlib import ExitStack

import numpy as np

import concourse.bass as bass
import concourse.tile as tile
from concourse import bacc, mybir
from concourse.bass_utils import run_bass_kernel_spmd

B, L, M = 64, 2048, 256
N_CORES = 8
BPC = B // N_CORES          # batches per core
LCH = L // 128              # 16 l-chunks per batch
RCH = M // 128              # 2 r-chunks (contraction over M)
TPACK = 8                   # transposes packed per (2-bank) PSUM tile
OPACK = 2                   # output l-chunks packed per PSUM bank

F32 = mybir.dt.float32
F32R = mybir.dt.float32r
BF16 = mybir.dt.bfloat16

# main gather matmul dtype: float32r streams at 1 cycle/row (vs 4 for
# float32) when the moving dim is >= 256; flip to False if hardware
# float32r precision is insufficient.
MAIN_F32R = True
# stream the transpose identity in bf16 (1 cycle/row vs 2 for f32; the
# 1.0 weights are exact in bf16). Requires f32r input tiles so the
# fp32-pairing assert passes.
FAST_T = True


def _consts():
    rng = np.arange(M, dtype=np.float64) - (M // 2) + 1.0          # -127..128
    denom = np.float64(np.float32(M) * np.float32(0.1))            # 25.6 as f32
    rng_over = (rng / denom).astype(np.float32).reshape(1, M)      # rng/25.6
    rmat = np.stack([np.ones(M), rng, rng * rng], axis=1).astype(np.float32)
    rmat = rmat.reshape(RCH, 128, 3)                               # [rc, r, k]
    iota = np.arange(128, dtype=np.float32)
    iota_cols = np.stack([iota + 128.0 * rc for rc in range(RCH)], axis=1)
    ident = np.eye(128, dtype=np.float32)
    ones_row = np.ones((1, 128), dtype=np.float32)
    return rng_over, rmat, iota_cols, ident, ones_row


def build_program(main_f32r=MAIN_F32R, fast_t=FAST_T, reps=1):
    nc = bacc.Bacc("TRN2", target_bir_lowering=False, debug=False)

    # declared f32r when fast_t: bit-identical to f32 (np dtype is float32),
    # avoids the DMA cast check while letting the PE stream it as f32r
    xdt_ = F32R if fast_t else F32
    x_dram = nc.dram_tensor("distance", [BPC, L, M], xdt_, kind="ExternalInput")
    out_dram = nc.dram_tensor("out", [BPC, L, M], F32, kind="ExternalOutput")

    rng_over, rmat, iota_cols, ident, ones_row = _consts()
    rng_dram = nc.inline_tensor(rng_over, "c_rng")
    rmat_dram = nc.inline_tensor(rmat, "c_rmat")
    iota_dram = nc.inline_tensor(iota_cols, "c_iota")
    ident_dram = nc.inline_tensor(ident, "c_ident")
    ones_dram = nc.inline_tensor(ones_row, "c_ones")

    mdt = F32R if main_f32r else F32
    xdt = F32R if fast_t else F32
    # the u broadcast must stay exact f32: u spans [0,256] and f32r's
    # ~12-bit effective mantissa would shift the gather index by ~0.03
    # bins, a ~3e-2 output error (measured on hardware)
    udt = F32

    with tile.TileContext(nc) as tc, ExitStack() as ctx:
        cpool = ctx.enter_context(tc.tile_pool(name="consts", bufs=1))
        xin_pool = ctx.enter_context(tc.tile_pool(name="xin", bufs=4))
        xt_pool = ctx.enter_context(tc.tile_pool(name="xt", bufs=3 * RCH))
        g_pool = ctx.enter_context(tc.tile_pool(name="g", bufs=2 * RCH))
        osb_pool = ctx.enter_context(tc.tile_pool(name="osb", bufs=3))
        st_pool = ctx.enter_context(tc.tile_pool(name="stats", bufs=2))
        ps_t = ctx.enter_context(tc.tile_pool(name="ps_t", bufs=2, space="PSUM"))
        ps_o = ctx.enter_context(tc.tile_pool(name="ps_o", bufs=3, space="PSUM"))
        # one bank shared by the u-broadcast [128,256] and the tiny stats
        # accumulator [1,3] (cols 256:259) — PSUM is only 8 banks
        ps_u = ctx.enter_context(tc.tile_pool(name="ps_u", bufs=1, space="PSUM"))

        c_rng = cpool.tile([1, M], F32, tag="c_rng")
        nc.scalar.dma_start(c_rng[:], rng_dram.ap())
        c_rmat = cpool.tile([128, RCH, 3], F32, tag="c_rmat")
        nc.scalar.dma_start(c_rmat[:], rmat_dram.ap().rearrange("rc r k -> r rc k"))
        c_iota = cpool.tile([128, RCH], F32, tag="c_iota")
        nc.scalar.dma_start(c_iota[:], iota_dram.ap())
        c_identf = cpool.tile([128, 128], F32, tag="c_identf")
        nc.scalar.dma_start(c_identf[:], ident_dram.ap())
        if fast_t:
            # f32r identity: walrus requires f32r matmul inputs to match
            # dtypes exactly (no f32r/bf16 mixing), and f32r streams the
            # transpose at 1.5 cycles/row vs 2.0 for f32
            c_ident = cpool.tile([128, 128], F32R, tag="c_ident")
            nc.vector.tensor_copy(c_ident[:], c_identf[:])
        else:
            c_ident = c_identf
        c_ones = cpool.tile([1, 128], F32, tag="c_ones")
        nc.scalar.dma_start(c_ones[:], ones_dram.ap())

        flip = [0]  # round-robin PSUM->SBUF copies across DVE and ACT

        n = BPC * reps
        blist = [b for _ in range(reps) for b in range(BPC)]

        # pipeline state per in-flight batch (keyed by position index)
        xin_t, xt_t, acc_t, pxt_t, st_t, u_t, g_t = {}, {}, {}, {}, {}, {}, {}

        def emit_load(i):
            # stripe loads across the SP HWDGE queue and the Pool SWDGE
            # queue: two transfers in flight doubles DMA-queue throughput.
            # The first two loads are split in half across BOTH queues so
            # the pipeline fills as early as possible.
            xin = xin_pool.tile([128, LCH, M], xdt, tag="xin")
            xr = x_dram.ap()[blist[i]].rearrange("(p lc) m -> p lc m", p=128)
            if i < 2:
                h = LCH // 2
                nc.sync.dma_start(xin[:, 0:h, :], xr[:, 0:h, :])
                nc.gpsimd.dma_start(xin[:, h:LCH, :], xr[:, h:LCH, :])
            else:
                eng = nc.sync if i % 2 == 0 else nc.gpsimd
                eng.dma_start(xin[:], xr[:])
            xin_t[i] = xin

        def emit_transpose(i):
            # xin (l on partitions) -> xt[rc] (m on partitions, 128 x 2048);
            # the packed PSUM->SBUF copies also emit row sums (px partials)
            xin = xin_t.pop(i)
            xt = [xt_pool.tile([128, L], mdt, tag="xt", name=f"xt{rc}")
                  for rc in range(RCH)]
            ngrp = LCH // TPACK
            acc = st_pool.tile([128, RCH, ngrp], F32, tag="acc")
            for j in range(ngrp):
                for rc in range(RCH):
                    tp = ps_t.tile([128, TPACK * 128], xdt, tag="tp")
                    for k in range(TPACK):
                        lc = TPACK * j + k
                        nc.tensor.transpose(
                            tp[:, 128 * k : 128 * (k + 1)],
                            xin[:, lc, 128 * rc : 128 * (rc + 1)],
                            c_ident[:],
                        )
                    dst = xt[rc][:, TPACK * 128 * j : TPACK * 128 * (j + 1)]
                    if flip[0] % 2 == 0:
                        nc.vector.tensor_scalar(
                            out=dst,
                            in0=tp[:],
                            scalar1=0.0,
                            scalar2=None,
                            op0=mybir.AluOpType.add,
                            op1=mybir.AluOpType.add,
                            accum_out=acc[:, rc, j : j + 1],
                        )
                    else:
                        nc.scalar.activation(
                            dst,
                            tp[:],
                            mybir.ActivationFunctionType.Copy,
                            accum_out=acc[:, rc, j : j + 1],
                        )
                    flip[0] += 1
            pxt = st_pool.tile([128, RCH], F32, tag="pxt")
            for rc in range(RCH):
                nc.vector.tensor_reduce(
                    out=pxt[:, rc : rc + 1],
                    in_=acc[:, rc, :],
                    axis=mybir.AxisListType.X,
                    op=mybir.AluOpType.add,
                )
            xt_t[i] = xt
            pxt_t[i] = pxt

        def emit_stats(i):
            # [S, T1, T2] = sum_r pxt[r] * [1, rng, rng^2]; then the scalar
            # chain down to u = clip(rng/25.6 * std + mean + 127, -1, 256)
            pxt = pxt_t.pop(i)
            ps_comb = ps_u.tile([128, M + 4], F32, tag="ps_comb")
            ps_stats = ps_comb[0:1, M : M + 3]
            for rc in range(RCH):
                nc.tensor.matmul(
                    ps_stats[:],
                    pxt[:, rc : rc + 1],
                    c_rmat[:, rc, :],
                    start=(rc == 0),
                    stop=(rc == RCH - 1),
                )
            st = st_pool.tile([1, 8], F32, tag="st")
            # st layout: 0:S 1:T1 2:T2 3:recipS 4:mean 5:m2 6:var 7:std
            nc.vector.tensor_scalar(
                out=st[:, 0:3], in0=ps_stats[:], scalar1=0.0, scalar2=None,
                op0=mybir.AluOpType.add,
            )
            nc.vector.reciprocal(st[:, 3:4], st[:, 0:1])
            nc.vector.tensor_mul(st[:, 4:5], st[:, 1:2], st[:, 3:4])
            nc.vector.tensor_mul(st[:, 5:6], st[:, 2:3], st[:, 3:4])
            # var = m2 - mean^2
            nc.vector.tensor_tensor(
                out=st[:, 6:7], in0=st[:, 4:5], in1=st[:, 4:5],
                op=mybir.AluOpType.mult,
            )
            nc.vector.tensor_sub(st[:, 6:7], st[:, 5:6], st[:, 6:7])
            nc.scalar.sqrt(st[:, 7:8], st[:, 6:7])
            meanp = st_pool.tile([1, 1], F32, tag="meanp")
            nc.vector.tensor_scalar_add(meanp[:], st[:, 4:5], float(M // 2 - 1))
            u_row = st_pool.tile([1, M], udt, tag="u_row")
            nc.vector.tensor_scalar(
                out=u_row[:], in0=c_rng[:],
                scalar1=st[:, 7:8], scalar2=meanp[:],
                op0=mybir.AluOpType.mult, op1=mybir.AluOpType.add,
            )
            nc.vector.tensor_scalar(
                out=u_row[:], in0=u_row[:],
                scalar1=-1.0, scalar2=float(M),
                op0=mybir.AluOpType.max, op1=mybir.AluOpType.min,
            )
            u_t[i] = u_row

        def emit_gbuild(i):
            # broadcast u across partitions; build -G = min(|iota-u|,1) - 1
            u_row = u_t.pop(i)
            ps_comb = ps_u.tile([128, M + 4], F32, tag="ps_comb", name="ps_comb_g")
            ps_ub = ps_comb[:, 0:M]
            nc.tensor.matmul(ps_ub, c_ones[:], u_row[:], start=True, stop=True)
            g = [g_pool.tile([128, M], mdt, tag="g", name=f"g{rc}")
                 for rc in range(RCH)]
            for rc in range(RCH):
                d = g_pool.tile([128, M], F32, tag="absd")
                nc.scalar.activation(
                    d[:], ps_ub[:], mybir.ActivationFunctionType.Abs,
                    bias=c_iota[:, rc : rc + 1], scale=-1.0,
                )
                nc.vector.tensor_scalar(
                    out=g[rc][:], in0=d[:],
                    scalar1=1.0, scalar2=1.0,
                    op0=mybir.AluOpType.min, op1=mybir.AluOpType.subtract,
                )
            g_t[i] = g

        def emit_main(i):
            # -out[lc] = sum_rc xt[rc][:,lc-chunk].T @ (-g[rc]); the
            # PSUM->SBUF copy multiplies by -1
            xt = xt_t.pop(i)
            g = g_t.pop(i)
            osb = osb_pool.tile([128, LCH, M], F32, tag="osb")
            for lo in range(LCH // OPACK):
                po = ps_o.tile([128, OPACK * M], F32, tag="po")
                for k in range(OPACK):
                    lc = OPACK * lo + k
                    for rc in range(RCH):
                        nc.tensor.matmul(
                            po[:, M * k : M * (k + 1)],
                            xt[rc][:, 128 * lc : 128 * (lc + 1)],
                            g[rc][:],
                            start=(rc == 0),
                            stop=(rc == RCH - 1),
                        )
                dst = osb[:, OPACK * lo : OPACK * (lo + 1), :]
                if flip[0] % 2 == 0:
                    nc.vector.tensor_scalar(
                        out=dst, in0=po[:],
                        scalar1=-1.0, scalar2=None, op0=mybir.AluOpType.mult,
                    )
                else:
                    nc.scalar.activation(
                        dst, po[:],
                        mybir.ActivationFunctionType.Copy, scale=-1.0,
                    )
                flip[0] += 1
            # stores stripe across the same two queues, opposite phase to
            # the loads so each queue carries 4 loads + 4 stores per rep.
            # The last two stores are split across both queues to shorten
            # the pipeline drain.
            orr = out_dram.ap()[blist[i]].rearrange("(p lc) m -> p lc m", p=128)
            if i >= n - 2:
                h = LCH // 2
                nc.sync.dma_start(orr[:, 0:h, :], osb[:, 0:h, :])
                nc.gpsimd.dma_start(orr[:, h:LCH, :], osb[:, h:LCH, :])
            else:
                eng = nc.sync if i % 2 == 0 else nc.gpsimd
                eng.dma_start(orr[:], osb[:])

        SKEW_STATS = 1  # segments between transpose and stats/gbuild
        SKEW_MAIN = 2   # segments between transpose and main/store
        for seg in range(n + SKEW_MAIN):
            i0 = seg
            i1 = seg - SKEW_STATS
            i2 = seg - SKEW_MAIN
            if i0 == 0:
                emit_load(0)
            if i0 + 1 < n:
                emit_load(i0 + 1)
            if 0 <= i1 < n and i1 < i0:
                emit_stats(i1)
            if i0 < n:
                emit_transpose(i0)
            if 0 <= i1 < n and i1 == i0:
                emit_stats(i1)
            if 0 <= i1 < n:
                emit_gbuild(i1)
            if 0 <= i2 < n:
                emit_main(i2)

    nc.compile()
    return nc


_NC_CACHE = None


def _get_program():
    global _NC_CACHE
    if _NC_CACHE is None:
        _NC_CACHE = build_program()
    return _NC_CACHE


def kernel(distance: np.ndarray) -> np.ndarray:
    assert distance.shape == (B, L, M), distance.shape
    x = np.ascontiguousarray(distance, dtype=np.float32)
    nc = _get_program()
    in_maps = [{"distance": x[i * BPC : (i + 1) * BPC]} for i in range(N_CORES)]
    res = run_bass_kernel_spmd(nc, in_maps, core_ids=list(range(N_CORES)))
    return np.concatenate([res.results[i]["out"] for i in range(N_CORES)], axis=0)

